# revision 28
# baseline (speedup 1.0000x reference)
"""Trainium2 Bass kernel for nn_Attention (dense transformer MHA forward).

Shapes: x [4096, 1024], 16 heads x head_dim 64, full softmax attention.

Sharding (8 cores, tensor-parallel over heads): each core owns 2 heads.
  - column-parallel qkv: core computes q,k,v for its 2 heads only
  - local attention for 2 heads
  - row-parallel proj: core computes a partial [4096, 1024] output
  - "all-reduce" = host-side sum of the 8 partials (+ b_proj once)

Device-kernel layout (per core, heads h0/h1), v2 schedule:
  - everything computed transposed: qkv^T [rows, seq]; S^T lands with seq-k on
    PSUM partitions and seq-q moving; softmax denominator produced by the PE
    via a ones-column folded into the stationary V operand of the P@V matmul.
  - exp() on the Scalar engine straight out of PSUM, 1/sqrt(head_dim) folded
    into the activation's free scale. No max-subtraction (scores ~N(0,1)).
  - v2: compute dtype bf16 (FWL weight loads), Q projected for ALL q-chunks
    before K/V so the scalar engine (the steady-state bottleneck) can run
    exp() for any (qc, kc) as soon as its K-chunk exists; S/exp run ahead of
    P@V consumption through a deep e-tile bank; proj/tail work is spread one
    piece per attention iteration to avoid per-qc burst stalls on PE/DVE.
"""

import numpy as np

SEQ = 4096
DIM = 1024
HEADS = 16
HD = 64
NCORES = 8
QCH = 512          # q-chunk (matmul moving free dim)
KCH = 128          # k-chunk (contraction tile)
NQ = SEQ // QCH    # 8
NK = SEQ // KCH    # 32
NDC = DIM // 128   # 8 contraction chunks for the qkv projection
ERUN = 14          # S/exp emission run-ahead over P@V consumption (e-bank)

_COMPILED = {}


def _build_nc(loop_n=None, cdt_name='bfloat16', staggered=False):
    import concourse.bass as bass
    import concourse.bacc as bacc
    from concourse import mybir, tile

    f32 = mybir.dt.float32
    f16 = mybir.dt.float16
    cdt = getattr(mybir.dt, cdt_name)
    nc = bacc.Bacc("TRN2", target_bir_lowering=False, debug=False)

    xT_d = nc.dram_tensor("xT", [NQ, 128, NDC, QCH], cdt, kind="ExternalInput")
    wqkvT_d = nc.dram_tensor("wqkvT", [128, NDC, 384], cdt, kind="ExternalInput")
    bq_d = nc.dram_tensor("bq", [128, 3], f32, kind="ExternalInput")
    wprojT_d = nc.dram_tensor("wprojT", [128, DIM], cdt, kind="ExternalInput")
    sel_d = nc.dram_tensor("sel", [128, 128], cdt, kind="ExternalInput")
    ident_d = nc.dram_tensor("ident", [128, 64], cdt, kind="ExternalInput")
    vfill_d = nc.dram_tensor("vfill", [128, NK, 65], cdt, kind="ExternalInput")
    zfill_d = nc.dram_tensor("zfill", [128, QCH], cdt, kind="ExternalInput")
    y_d = nc.dram_tensor("y", [SEQ, DIM], f32, kind="ExternalOutput")

    EXP = mybir.ActivationFunctionType.Exp

    with tile.TileContext(nc) as tc, nc.allow_low_precision(
        reason="bf16 matmul inputs, fp32 PSUM accumulate; tolerance 2e-2"
    ):
        with (
            tc.tile_pool(name="const", bufs=1) as const,
            tc.tile_pool(name="xpool", bufs=NQ) as xpool,
            tc.tile_pool(name="big", bufs=1) as big,
            tc.tile_pool(name="epool", bufs=ERUN + 2) as epool,
            tc.tile_pool(name="opool", bufs=2) as opool,
            tc.tile_pool(name="ypool", bufs=3) as ypool,
            tc.tile_pool(name="spsum", bufs=2, space="PSUM") as spsum,
            tc.tile_pool(name="opsum", bufs=1, space="PSUM") as opsum,
            tc.tile_pool(name="mpsum", bufs=2, space="PSUM") as mpsum,
        ):
            mtag = "mm"
            # ---- constants ----
            wq = const.tile([128, NDC, 384], cdt)
            nc.sync.dma_start(out=wq, in_=wqkvT_d.ap())
            wp = const.tile([128, DIM], cdt)
            nc.sync.dma_start(out=wp, in_=wprojT_d.ap())
            bq = const.tile([128, 3], f32)
            nc.sync.dma_start(out=bq, in_=bq_d.ap())
            sel = const.tile([128, 128], cdt)
            nc.sync.dma_start(out=sel, in_=sel_d.ap())
            idn = const.tile([128, 64], cdt)
            nc.sync.dma_start(out=idn, in_=ident_d.ap())

            # ---- persistent SBUF state ----
            KT = big.tile([128, SEQ], cdt)   # rows 0:64 K^T h0, 64:128 K^T h1
            VT = big.tile([128, SEQ], cdt)
            QT = big.tile([128, SEQ], cdt)
            # per k-chunk stationary for P@V:
            #   cols 0:64 V_h0 | 64 ones | then h1 slab (65:193):
            #   local [0:32] zeros | [32] ones | [33:64] zeros | [64:128] V_h1
            #   so h1's Z lands on PSUM partition 32 (32-aligned APs only)
            vall = big.tile([128, NK, 193], cdt)
            zsb = big.tile([128, QCH], cdt)  # softmax-recip staging rows 63/64

            nc.sync.dma_start(out=zsb, in_=zfill_d.ap())
            nc.sync.dma_start(out=vall[:, :, 64:129], in_=vfill_d.ap())

            import contextlib
            loop_cm = (
                tc.For_i(0, loop_n, 1, staggered_reset=staggered,
                         hint_engines=(
                             mybir.EngineType.PE, mybir.EngineType.DVE,
                             mybir.EngineType.Activation, mybir.EngineType.SP,
                             mybir.EngineType.Pool,
                         ))
                if loop_n else contextlib.nullcontext()
            )
            with loop_cm:
                xTr = xT_d.ap()  # host-tiled [sc, 128, dc, q] for contiguous DMA
                xs_t = []
                for sc in range(NQ):
                    t = xpool.tile([128, NDC, QCH], cdt, tag="xs", name="xs")
                    nc.sync.dma_start(out=t, in_=xTr[sc])
                    xs_t.append(t)

                # keep the PE HAM warm across the loop back-edge + xs DMA
                # wait: a burst of dependency-free matmuls on resident data
                wu = mpsum.tile([128, QCH], f32, tag=mtag, name="wu")
                for _ in range(6):
                    nc.tensor.matmul(wu, lhsT=wq[:, 0, 0:128], rhs=zsb,
                                     start=True, stop=True)

                def emit_mm_block(sc, m, dest):  # m: 0=K, 1=V, 2=Q
                    ps = mpsum.tile([128, QCH], f32, tag=mtag, name="ps")
                    for dc in range(NDC):
                        nc.tensor.matmul(
                            ps,
                            lhsT=wq[:, dc, m * 128:(m + 1) * 128],
                            rhs=xs_t[sc][:, dc, :],
                            start=(dc == 0),
                            stop=(dc == NDC - 1),
                        )
                    nc.vector.tensor_scalar_add(
                        dest[:, sc * QCH:(sc + 1) * QCH], ps, bq[:, m:m + 1]
                    )

                def emit_vtrans(sc):
                    for kc in range(4 * sc, 4 * sc + 4):
                        for h in range(2):
                            tp = mpsum.tile([128, 64], cdt, tag=mtag, name="tp")
                            nc.tensor.transpose(
                                tp, VT[64 * h:64 * h + 64, kc * 128:(kc + 1) * 128],
                                idn[64 * h:64 * h + 64, :]
                            )
                            dst = 0 if h == 0 else 129
                            nc.vector.tensor_copy(vall[:, kc, dst:dst + 64], tp)

                # ---- attention state: S/exp stream runs ahead of P@V ----
                SEQS = [(qc, kc) for qc in range(NQ) for kc in range(NK)]
                NTOT = len(SEQS)
                e_tiles = {}
                st = {"s": 0, "pv": 0, "frontier": -1,
                      "o0": None, "o1": None, "proj": []}

                def pump_s():
                    while st["s"] < NTOT and st["s"] - st["pv"] < ERUN:
                        qc, kc = SEQS[st["s"]]
                        if qc > st["frontier"] or kc >= 4 * (st["frontier"] + 1):
                            return
                        qsl = slice(qc * QCH, (qc + 1) * QCH)
                        ksl = slice(kc * 128, (kc + 1) * 128)
                        s_ps = spsum.tile([128, 2 * QCH], f32, tag="s", name="s_ps")
                        nc.tensor.matmul(
                            s_ps[:, 0:QCH], lhsT=KT[0:64, ksl], rhs=QT[0:64, qsl],
                            start=True, stop=True,
                        )
                        nc.tensor.matmul(
                            s_ps[:, QCH:2 * QCH], lhsT=KT[64:128, ksl],
                            rhs=QT[64:128, qsl],
                            start=True, stop=True,
                        )
                        e = epool.tile([128, 2 * QCH], cdt, tag="e", name="e")
                        nc.scalar.activation(e, s_ps, EXP, scale=1.0 / np.sqrt(HD))
                        e_tiles[st["s"]] = e
                        st["s"] += 1

                def emit_tail(o0, o1):
                    # softmax denominators: Z0 at o0 row 64, Z1 at o1 row 32
                    nc.vector.reciprocal(zsb[64:65, :], o0[64:65, :])
                    nc.vector.reciprocal(zsb[32:33, :], o1[32:33, :])
                    zb = mpsum.tile([128, QCH], f32, tag=mtag, name="zb")
                    nc.tensor.matmul(zb, lhsT=sel, rhs=zsb, start=True, stop=True)
                    zbs = opool.tile([128, QCH], f32, tag="zbs", name="zbs")
                    nc.vector.tensor_copy(zbs, zb)
                    ot = opool.tile([128, QCH], cdt, tag="ot", name="ot")
                    nc.vector.tensor_mul(ot[0:64, :], o0[0:64, :], zbs[0:64, :])
                    nc.vector.tensor_mul(ot[64:128, :], o1[64:128, :],
                                         zbs[64:128, :])
                    return ot

                def make_proj(ot, qc):
                    # 12 emission units: (alloc+mm, mm, dma) x 4 row-blocks
                    units = []
                    box = {}

                    def mk_mm(ss, oh, first):
                        def f():
                            if first:
                                box[ss] = ypool.tile([128, DIM], f32, tag="y",
                                                     name="ysb")
                            yp = mpsum.tile([128, QCH], f32, tag=mtag, name="yp")
                            nc.tensor.matmul(
                                yp,
                                lhsT=ot[:, ss * 128:(ss + 1) * 128],
                                rhs=wp[:, oh * QCH:(oh + 1) * QCH],
                                start=True, stop=True,
                            )
                            nc.vector.tensor_copy(
                                box[ss][:, oh * QCH:(oh + 1) * QCH], yp)
                        return f

                    def mk_dma(ss):
                        def f():
                            r0 = qc * QCH + ss * 128
                            nc.sync.dma_start(out=y_d.ap()[r0:r0 + 128, :],
                                              in_=box[ss])
                        return f

                    for ss in range(4):
                        units.append(mk_mm(ss, 0, True))
                        units.append(mk_mm(ss, 1, False))
                        units.append(mk_dma(ss))
                    return units

                def advance(max_pv):
                    for _ in range(max_pv):
                        pump_s()
                        if st["pv"] >= NTOT or st["pv"] >= st["s"]:
                            return
                        qc, kc = SEQS[st["pv"]]
                        if kc == 0:
                            if st["o0"] is not None:
                                ot = emit_tail(st["o0"], st["o1"])
                                st["proj"] = make_proj(ot, qc - 1)
                            st["o0"] = opsum.tile([128, QCH], f32, tag="o0",
                                                  name="o0")
                            st["o1"] = opsum.tile([128, QCH], f32, tag="o1",
                                                  name="o1")
                        e = e_tiles.pop(st["pv"])
                        nc.tensor.matmul(
                            st["o0"][0:65, :], lhsT=vall[:, kc, 0:65],
                            rhs=e[:, 0:QCH],
                            start=(kc == 0), stop=(kc == NK - 1),
                        )
                        nc.tensor.matmul(
                            st["o1"], lhsT=vall[:, kc, 65:193],
                            rhs=e[:, QCH:2 * QCH],
                            start=(kc == 0), stop=(kc == NK - 1),
                        )
                        st["pv"] += 1
                        if staggered and loop_n and st["pv"] in (64, 128, 192):
                            # stage boundaries at q-chunk edges: stage 3
                            # (qc6-7 + drain) overlaps the next iteration's
                            # DMA/qkv stage 0 under staggered reset
                            tc.stage_boundary()
                        if st["proj"]:
                            st["proj"].pop(0)()

                # ---- emission: Q first per sc, then K/V; attention chases ----
                for sc in range(NQ):
                    emit_mm_block(sc, 2, QT)   # Q for q-chunk sc
                    emit_mm_block(sc, 0, KT)
                    emit_mm_block(sc, 1, VT)
                    emit_vtrans(sc)
                    st["frontier"] = sc
                    advance(4)

                while st["pv"] < NTOT or st["proj"]:
                    if st["pv"] >= NTOT:
                        st["proj"].pop(0)()
                    else:
                        advance(1)

                # final q-chunk tail + proj
                ot = emit_tail(st["o0"], st["o1"])
                for u in make_proj(ot, NQ - 1):
                    u()

    nc.compile()
    return nc


def _cdt_np(a):
    if CDT == "float16":
        return np.ascontiguousarray(a).astype(np.float16)
    if CDT == "bfloat16":
        import ml_dtypes
        return np.ascontiguousarray(a).astype(ml_dtypes.bfloat16)
    # float32r: round-to-nearest-even at 11-bit mantissa, stored as f32
    b = np.ascontiguousarray(a).view(np.uint32)
    lsb = (b >> np.uint32(12)) & np.uint32(1)
    out = (b + np.uint32(0x7FF) + lsb) & np.uint32(0xFFFFF000)
    return out.view(np.float32)


def _prep_inputs(x, W_qkv, b_qkv, W_proj):
    """Host-side shard prep. Returns per-core input maps for the SPMD kernel."""
    # [sc, p, dc, q] layout: xt[sc, p, dc, q] = x[sc*512+q, dc*128+p]
    xT = _cdt_np(np.ascontiguousarray(
        x.reshape(NQ, QCH, NDC, 128).transpose(0, 3, 2, 1)))
    sel = np.zeros((128, 128), dtype=np.float32)
    sel[64, 0:64] = 1.0  # zsb partition 64 (recip Z0) -> bcast rows 0:64
    sel[32, 64:128] = 1.0  # zsb partition 32 (recip Z1) -> bcast rows 64:128
    sel = _cdt_np(sel)
    ident = _cdt_np(np.ascontiguousarray(np.vstack([np.eye(64, dtype=np.float32)] * 2)))
    patt = np.zeros(65, dtype=np.float32)
    patt[0] = 1.0   # vall col 64: ones column for head 0 sums
    patt[33] = 1.0  # vall col 97: ones column for head 1 sums (partition 32)
    vfill = _cdt_np(np.ascontiguousarray(np.broadcast_to(patt, (128, NK, 65))))
    zfill = _cdt_np(np.zeros((128, QCH), dtype=np.float32))

    in_maps = []
    for c in range(NCORES):
        h0 = 2 * c
        idx = np.concatenate([
            np.arange(DIM + HD * h0, DIM + HD * h0 + 128),          # K rows
            np.arange(2 * DIM + HD * h0, 2 * DIM + HD * h0 + 128),  # V rows
            np.arange(HD * h0, HD * h0 + 128),                      # Q rows
        ])
        w_shard = W_qkv[idx]                                  # [384, 1024]
        # [p, dc, row]: wq[p, dc, r] = w_shard[r, dc*128+p]
        wqkvT = _cdt_np(np.ascontiguousarray(
            w_shard.T.reshape(NDC, 128, 384).transpose(1, 0, 2)))
        bq = np.ascontiguousarray(b_qkv[idx].reshape(3, 128).T)  # [128, 3]
        wprojT = _cdt_np(np.ascontiguousarray(W_proj[:, 128 * c:128 * (c + 1)].T))
        in_maps.append({
            "xT": xT,
            "wqkvT": wqkvT,
            "bq": bq,
            "wprojT": wprojT,
            "sel": sel,
            "ident": ident,
            "vfill": vfill,
            "zfill": zfill,
        })
    return in_maps


CDT = "bfloat16"
STAGGERED = True


def _get_nc(loop_n=None):
    key = ("nc", loop_n, CDT, STAGGERED)
    if key not in _COMPILED:
        _COMPILED[key] = _build_nc(loop_n, cdt_name=CDT, staggered=STAGGERED)
    return _COMPILED[key]


def run(x, W_qkv, b_qkv, W_proj, b_proj, trace=False, **trace_kwargs):
    """Run the sharded kernel; returns (y_full, BassKernelResults)."""
    from concourse.bass_utils import run_bass_kernel_spmd

    x = np.asarray(x, dtype=np.float32)
    W_qkv = np.asarray(W_qkv, dtype=np.float32)
    b_qkv = np.asarray(b_qkv, dtype=np.float32)
    W_proj = np.asarray(W_proj, dtype=np.float32)
    b_proj = np.asarray(b_proj, dtype=np.float32)

    nc = _get_nc()
    in_maps = _prep_inputs(x, W_qkv, b_qkv, W_proj)
    res = run_bass_kernel_spmd(
        nc, in_maps, core_ids=list(range(NCORES)), trace=trace, **trace_kwargs
    )
    y = np.zeros((SEQ, DIM), dtype=np.float32)
    for r in res.results:
        y += r["y"].astype(np.float32)
    y += b_proj
    return y, res


def kernel(x, W_qkv, b_qkv, W_proj, b_proj):
    y, _ = run(x, W_qkv, b_qkv, W_proj, b_proj, trace=False)
    return y


# revision 29
# speedup vs baseline: 1.0475x; 1.0475x over previous
"""Trainium2 Bass kernel for nn_Attention (dense transformer MHA forward).

Shapes: x [4096, 1024], 16 heads x head_dim 64, full softmax attention.

Sharding (8 cores, tensor-parallel over heads): each core owns 2 heads.
  - column-parallel qkv: core computes q,k,v for its 2 heads only
  - local attention for 2 heads
  - row-parallel proj: core computes a partial [4096, 1024] output
  - "all-reduce" = host-side sum of the 8 partials (+ b_proj once)

Device-kernel layout (per core, heads h0/h1), v2 schedule:
  - everything computed transposed: qkv^T [rows, seq]; S^T lands with seq-k on
    PSUM partitions and seq-q moving; softmax denominator produced by the PE
    via a ones-column folded into the stationary V operand of the P@V matmul.
  - exp() on the Scalar engine straight out of PSUM, 1/sqrt(head_dim) folded
    into the activation's free scale. No max-subtraction (scores ~N(0,1)).
  - v2: compute dtype bf16 (FWL weight loads), Q projected for ALL q-chunks
    before K/V so the scalar engine (the steady-state bottleneck) can run
    exp() for any (qc, kc) as soon as its K-chunk exists; S/exp run ahead of
    P@V consumption through a deep e-tile bank; proj/tail work is spread one
    piece per attention iteration to avoid per-qc burst stalls on PE/DVE.
"""

import numpy as np

SEQ = 4096
DIM = 1024
HEADS = 16
HD = 64
NCORES = 8
QCH = 512          # q-chunk (matmul moving free dim)
KCH = 128          # k-chunk (contraction tile)
NQ = SEQ // QCH    # 8
NK = SEQ // KCH    # 32
NDC = DIM // 128   # 8 contraction chunks for the qkv projection
ERUN = 14          # S/exp emission run-ahead over P@V consumption (e-bank)

_COMPILED = {}


def _build_nc(loop_n=None, cdt_name='bfloat16', staggered=False):
    import concourse.bass as bass
    import concourse.bacc as bacc
    from concourse import mybir, tile

    f32 = mybir.dt.float32
    f16 = mybir.dt.float16
    cdt = getattr(mybir.dt, cdt_name)
    nc = bacc.Bacc("TRN2", target_bir_lowering=False, debug=False)

    xT_d = nc.dram_tensor("xT", [NQ, 128, NDC, QCH], cdt, kind="ExternalInput")
    wqkvT_d = nc.dram_tensor("wqkvT", [128, NDC, 384], cdt, kind="ExternalInput")
    bq_d = nc.dram_tensor("bq", [128, 3], f32, kind="ExternalInput")
    wprojT_d = nc.dram_tensor("wprojT", [128, DIM], cdt, kind="ExternalInput")
    sel_d = nc.dram_tensor("sel", [128, 128], cdt, kind="ExternalInput")
    ident_d = nc.dram_tensor("ident", [128, 64], cdt, kind="ExternalInput")
    vfill_d = nc.dram_tensor("vfill", [128, NK, 65], cdt, kind="ExternalInput")
    zfill_d = nc.dram_tensor("zfill", [128, QCH], cdt, kind="ExternalInput")
    y_d = nc.dram_tensor("y", [SEQ, DIM], f32, kind="ExternalOutput")

    EXP = mybir.ActivationFunctionType.Exp

    with tile.TileContext(nc) as tc, nc.allow_low_precision(
        reason="bf16 matmul inputs, fp32 PSUM accumulate; tolerance 2e-2"
    ):
        with (
            tc.tile_pool(name="const", bufs=1) as const,
            tc.tile_pool(name="xpool", bufs=NQ) as xpool,
            tc.tile_pool(name="big", bufs=1) as big,
            tc.tile_pool(name="epool", bufs=ERUN + 2) as epool,
            tc.tile_pool(name="opool", bufs=2) as opool,
            tc.tile_pool(name="ypool", bufs=3) as ypool,
            tc.tile_pool(name="spsum", bufs=2, space="PSUM") as spsum,
            tc.tile_pool(name="opsum", bufs=1, space="PSUM") as opsum,
            tc.tile_pool(name="mpsum", bufs=2, space="PSUM") as mpsum,
        ):
            mtag = "mm"
            # ---- constants ----
            wq = const.tile([128, NDC, 384], cdt)
            nc.sync.dma_start(out=wq, in_=wqkvT_d.ap())
            wp = const.tile([128, DIM], cdt)
            nc.sync.dma_start(out=wp, in_=wprojT_d.ap())
            bq = const.tile([128, 3], f32)
            nc.sync.dma_start(out=bq, in_=bq_d.ap())
            sel = const.tile([128, 128], cdt)
            nc.sync.dma_start(out=sel, in_=sel_d.ap())
            idn = const.tile([128, 64], cdt)
            nc.sync.dma_start(out=idn, in_=ident_d.ap())

            # ---- persistent SBUF state ----
            KT = big.tile([128, SEQ], cdt)   # rows 0:64 K^T h0, 64:128 K^T h1
            VT = big.tile([128, SEQ], cdt)
            QT = big.tile([128, SEQ], cdt)
            # per k-chunk stationary for P@V:
            #   cols 0:64 V_h0 | 64 ones | then h1 slab (65:193):
            #   local [0:32] zeros | [32] ones | [33:64] zeros | [64:128] V_h1
            #   so h1's Z lands on PSUM partition 32 (32-aligned APs only)
            vall = big.tile([128, NK, 193], cdt)
            zsb = big.tile([128, QCH], cdt)  # softmax-recip staging rows 63/64

            nc.sync.dma_start(out=zsb, in_=zfill_d.ap())
            nc.sync.dma_start(out=vall[:, :, 64:129], in_=vfill_d.ap())

            import contextlib
            loop_cm = (
                tc.For_i(0, loop_n, 1, staggered_reset=staggered,
                         hint_engines=(
                             mybir.EngineType.PE, mybir.EngineType.DVE,
                             mybir.EngineType.Activation, mybir.EngineType.SP,
                             mybir.EngineType.Pool,
                         ))
                if loop_n else contextlib.nullcontext()
            )
            with loop_cm:
                xTr = xT_d.ap()  # host-tiled [sc, 128, dc, q] for contiguous DMA
                xs_t = []
                for sc in range(NQ):
                    t = xpool.tile([128, NDC, QCH], cdt, tag="xs", name="xs")
                    nc.sync.dma_start(out=t, in_=xTr[sc])
                    xs_t.append(t)

                # keep the PE HAM warm across the loop back-edge + xs DMA
                # wait: a burst of dependency-free matmuls on resident data
                wu = mpsum.tile([128, QCH], f32, tag=mtag, name="wu")
                for _ in range(6):
                    nc.tensor.matmul(wu, lhsT=wq[:, 0, 0:128], rhs=zsb,
                                     start=True, stop=True)

                def emit_mm_block(sc, m, dest):  # m: 0=K, 1=V, 2=Q
                    ps = mpsum.tile([128, QCH], f32, tag=mtag, name="ps")
                    for dc in range(NDC):
                        nc.tensor.matmul(
                            ps,
                            lhsT=wq[:, dc, m * 128:(m + 1) * 128],
                            rhs=xs_t[sc][:, dc, :],
                            start=(dc == 0),
                            stop=(dc == NDC - 1),
                        )
                    nc.vector.tensor_scalar_add(
                        dest[:, sc * QCH:(sc + 1) * QCH], ps, bq[:, m:m + 1]
                    )

                def emit_vtrans(sc):
                    for kc in range(4 * sc, 4 * sc + 4):
                        for h in range(2):
                            tp = mpsum.tile([128, 64], cdt, tag=mtag, name="tp")
                            nc.tensor.transpose(
                                tp, VT[64 * h:64 * h + 64, kc * 128:(kc + 1) * 128],
                                idn[64 * h:64 * h + 64, :]
                            )
                            dst = 0 if h == 0 else 129
                            nc.vector.tensor_copy(vall[:, kc, dst:dst + 64], tp)

                # ---- attention state: S/exp stream runs ahead of P@V ----
                SEQS = [(qc, kc) for qc in range(NQ) for kc in range(NK)]
                NTOT = len(SEQS)
                e_tiles = {}
                st = {"s": 0, "pv": 0, "frontier": -1,
                      "o0": None, "o1": None, "proj": []}

                def pump_s():
                    while st["s"] < NTOT and st["s"] - st["pv"] < ERUN:
                        qc, kc = SEQS[st["s"]]
                        if qc > st["frontier"] or kc >= 4 * (st["frontier"] + 1):
                            return
                        qsl = slice(qc * QCH, (qc + 1) * QCH)
                        ksl = slice(kc * 128, (kc + 1) * 128)
                        s_ps = spsum.tile([128, 2 * QCH], f32, tag="s", name="s_ps")
                        nc.tensor.matmul(
                            s_ps[:, 0:QCH], lhsT=KT[0:64, ksl], rhs=QT[0:64, qsl],
                            start=True, stop=True,
                        )
                        nc.tensor.matmul(
                            s_ps[:, QCH:2 * QCH], lhsT=KT[64:128, ksl],
                            rhs=QT[64:128, qsl],
                            start=True, stop=True,
                        )
                        e = epool.tile([128, 2 * QCH], cdt, tag="e", name="e")
                        nc.scalar.activation(e, s_ps, EXP, scale=1.0 / np.sqrt(HD))
                        e_tiles[st["s"]] = e
                        st["s"] += 1

                def emit_tail(o0, o1):
                    # softmax denominators: Z0 at o0 row 64, Z1 at o1 row 32
                    nc.vector.reciprocal(zsb[64:65, :], o0[64:65, :])
                    nc.vector.reciprocal(zsb[32:33, :], o1[32:33, :])
                    zb = mpsum.tile([128, QCH], f32, tag=mtag, name="zb")
                    nc.tensor.matmul(zb, lhsT=sel, rhs=zsb, start=True, stop=True)
                    zbs = opool.tile([128, QCH], f32, tag="zbs", name="zbs")
                    nc.vector.tensor_copy(zbs, zb)
                    ot = opool.tile([128, QCH], cdt, tag="ot", name="ot")
                    nc.vector.tensor_mul(ot[0:64, :], o0[0:64, :], zbs[0:64, :])
                    nc.vector.tensor_mul(ot[64:128, :], o1[64:128, :],
                                         zbs[64:128, :])
                    return ot

                def make_proj(ot, qc):
                    # 12 emission units: (alloc+mm, mm, dma) x 4 row-blocks
                    units = []
                    box = {}

                    def mk_mm(ss, oh, first):
                        def f():
                            if first:
                                box[ss] = ypool.tile([128, DIM], f32, tag="y",
                                                     name="ysb")
                            yp = mpsum.tile([128, QCH], f32, tag=mtag, name="yp")
                            nc.tensor.matmul(
                                yp,
                                lhsT=ot[:, ss * 128:(ss + 1) * 128],
                                rhs=wp[:, oh * QCH:(oh + 1) * QCH],
                                start=True, stop=True,
                            )
                            nc.vector.tensor_copy(
                                box[ss][:, oh * QCH:(oh + 1) * QCH], yp)
                        return f

                    def mk_dma(ss):
                        def f():
                            r0 = qc * QCH + ss * 128
                            nc.sync.dma_start(out=y_d.ap()[r0:r0 + 128, :],
                                              in_=box[ss])
                        return f

                    for ss in range(4):
                        units.append(mk_mm(ss, 0, True))
                        units.append(mk_mm(ss, 1, False))
                        units.append(mk_dma(ss))
                    return units

                def advance(max_pv):
                    for _ in range(max_pv):
                        pump_s()
                        if st["pv"] >= NTOT or st["pv"] >= st["s"]:
                            return
                        qc, kc = SEQS[st["pv"]]
                        if kc == 0:
                            if st["o0"] is not None:
                                ot = emit_tail(st["o0"], st["o1"])
                                st["proj"] = make_proj(ot, qc - 1)
                            st["o0"] = opsum.tile([128, QCH], f32, tag="o0",
                                                  name="o0")
                            st["o1"] = opsum.tile([128, QCH], f32, tag="o1",
                                                  name="o1")
                        e = e_tiles.pop(st["pv"])
                        nc.tensor.matmul(
                            st["o0"][0:65, :], lhsT=vall[:, kc, 0:65],
                            rhs=e[:, 0:QCH],
                            start=(kc == 0), stop=(kc == NK - 1),
                        )
                        nc.tensor.matmul(
                            st["o1"], lhsT=vall[:, kc, 65:193],
                            rhs=e[:, QCH:2 * QCH],
                            start=(kc == 0), stop=(kc == NK - 1),
                        )
                        st["pv"] += 1
                        if staggered and loop_n and st["pv"] in (64, 128, 192):
                            # stage boundaries at q-chunk edges: stage 3
                            # (qc6-7 + drain) overlaps the next iteration's
                            # DMA/qkv stage 0 under staggered reset
                            tc.stage_boundary()
                        if st["proj"]:
                            st["proj"].pop(0)()

                # ---- emission: Q first per sc, then K/V; attention chases ----
                for sc in range(NQ):
                    emit_mm_block(sc, 2, QT)   # Q for q-chunk sc
                    emit_mm_block(sc, 0, KT)
                    emit_mm_block(sc, 1, VT)
                    emit_vtrans(sc)
                    st["frontier"] = sc
                    advance(4)

                while st["pv"] < NTOT or st["proj"]:
                    if st["pv"] >= NTOT:
                        st["proj"].pop(0)()
                    else:
                        advance(1)

                # final q-chunk tail + proj
                ot = emit_tail(st["o0"], st["o1"])
                for u in make_proj(ot, NQ - 1):
                    u()

    nc.compile()
    return nc


def _cdt_np(a):
    if CDT == "float16":
        return np.ascontiguousarray(a).astype(np.float16)
    if CDT == "bfloat16":
        import ml_dtypes
        return np.ascontiguousarray(a).astype(ml_dtypes.bfloat16)
    # float32r: round-to-nearest-even at 11-bit mantissa, stored as f32
    b = np.ascontiguousarray(a).view(np.uint32)
    lsb = (b >> np.uint32(12)) & np.uint32(1)
    out = (b + np.uint32(0x7FF) + lsb) & np.uint32(0xFFFFF000)
    return out.view(np.float32)


def _prep_inputs(x, W_qkv, b_qkv, W_proj):
    """Host-side shard prep. Returns per-core input maps for the SPMD kernel."""
    # [sc, p, dc, q] layout: xt[sc, p, dc, q] = x[sc*512+q, dc*128+p]
    xT = _cdt_np(np.ascontiguousarray(
        x.reshape(NQ, QCH, NDC, 128).transpose(0, 3, 2, 1)))
    sel = np.zeros((128, 128), dtype=np.float32)
    sel[64, 0:64] = 1.0  # zsb partition 64 (recip Z0) -> bcast rows 0:64
    sel[32, 64:128] = 1.0  # zsb partition 32 (recip Z1) -> bcast rows 64:128
    sel = _cdt_np(sel)
    ident = _cdt_np(np.ascontiguousarray(np.vstack([np.eye(64, dtype=np.float32)] * 2)))
    patt = np.zeros(65, dtype=np.float32)
    patt[0] = 1.0   # vall col 64: ones column for head 0 sums
    patt[33] = 1.0  # vall col 97: ones column for head 1 sums (partition 32)
    vfill = _cdt_np(np.ascontiguousarray(np.broadcast_to(patt, (128, NK, 65))))
    zfill = _cdt_np(np.zeros((128, QCH), dtype=np.float32))

    in_maps = []
    for c in range(NCORES):
        h0 = 2 * c
        idx = np.concatenate([
            np.arange(DIM + HD * h0, DIM + HD * h0 + 128),          # K rows
            np.arange(2 * DIM + HD * h0, 2 * DIM + HD * h0 + 128),  # V rows
            np.arange(HD * h0, HD * h0 + 128),                      # Q rows
        ])
        w_shard = W_qkv[idx]                                  # [384, 1024]
        # [p, dc, row]: wq[p, dc, r] = w_shard[r, dc*128+p]
        wqkvT = _cdt_np(np.ascontiguousarray(
            w_shard.T.reshape(NDC, 128, 384).transpose(1, 0, 2)))
        bq = np.ascontiguousarray(b_qkv[idx].reshape(3, 128).T)  # [128, 3]
        wprojT = _cdt_np(np.ascontiguousarray(W_proj[:, 128 * c:128 * (c + 1)].T))
        in_maps.append({
            "xT": xT,
            "wqkvT": wqkvT,
            "bq": bq,
            "wprojT": wprojT,
            "sel": sel,
            "ident": ident,
            "vfill": vfill,
            "zfill": zfill,
        })
    return in_maps


CDT = "bfloat16"
STAGGERED = False


def _get_nc(loop_n=None):
    key = ("nc", loop_n, CDT, STAGGERED)
    if key not in _COMPILED:
        _COMPILED[key] = _build_nc(loop_n, cdt_name=CDT, staggered=STAGGERED)
    return _COMPILED[key]


def run(x, W_qkv, b_qkv, W_proj, b_proj, trace=False, **trace_kwargs):
    """Run the sharded kernel; returns (y_full, BassKernelResults)."""
    from concourse.bass_utils import run_bass_kernel_spmd

    x = np.asarray(x, dtype=np.float32)
    W_qkv = np.asarray(W_qkv, dtype=np.float32)
    b_qkv = np.asarray(b_qkv, dtype=np.float32)
    W_proj = np.asarray(W_proj, dtype=np.float32)
    b_proj = np.asarray(b_proj, dtype=np.float32)

    nc = _get_nc()
    in_maps = _prep_inputs(x, W_qkv, b_qkv, W_proj)
    res = run_bass_kernel_spmd(
        nc, in_maps, core_ids=list(range(NCORES)), trace=trace, **trace_kwargs
    )
    y = np.zeros((SEQ, DIM), dtype=np.float32)
    for r in res.results:
        y += r["y"].astype(np.float32)
    y += b_proj
    return y, res


def kernel(x, W_qkv, b_qkv, W_proj, b_proj):
    y, _ = run(x, W_qkv, b_qkv, W_proj, b_proj, trace=False)
    return y
